# revision 68
# baseline (speedup 1.0000x reference)
"""MultiHeadAttention Trainium2 Bass kernel (v3, fp16, head-sharded).

Model: B=2, S=2048, D_MODEL=1024, H=16 heads, Dh=64.
  q/k/v = x @ W.T + b ; scores = (q k^T)/8 masked-softmax ; out = w @ v ; y = out @ Wy.T + by

Sharding (per the tensor-parallel head sharding): core c handles batch
b = c // 4 and head group g = c % 4 (4 heads), over ALL 2048 queries.
Wq/Wk/Wv are column-sharded, Wy row-sharded: each core emits a PARTIAL
y (fp16); kernel() sums the 4 partials per batch on the host and adds by.
This avoids the 4x duplicated K/V projection of query-sharding and cuts
per-core matmul cycles from ~591k to ~394k — which matters doubly here
because the board clamps the PE clock to 1.2GHz after ~150us of
sustained matmul activity.

Other design points:
  - all matmul operands fp16 (fp32 PSUM accum); V stays in SBUF
  - mask applied multiplicatively AFTER exp (exp reads PSUM on Act,
    fp16 mask-multiply split across DVE and GpSimd)
  - per-(head, query-block) softmax normalization: denominator row moved
    to partition 0, reciprocal_approx_fast, broadcast to 64 partitions
    via a ones-vector matmul, multiply fused with the PSUM eviction;
    the whole tail is deferred behind the next block's first scores
    matmuls so it never stalls the PE queue
"""

import numpy as np

import concourse.bass as bass
import concourse.mybir as mybir
import concourse.tile as tile
from concourse import bacc
from concourse.bass_utils import run_bass_kernel_spmd

F32 = mybir.dt.float32
F16 = mybir.dt.float16
I8 = mybir.dt.int8
NP16 = np.float16

B, S, D, H, DH = 2, 2048, 1024, 16, 64
QS = 512
P = 128
KO = D // P    # 8 contraction chunks over D
NKT = S // P   # 16 key chunks
HL = 4         # local heads per core
DL = HL * DH   # 256 local head dims
ML = DL // P   # 2 local m-blocks
NVH = HL * (S // QS)  # 16 virtual heads = (local head, query block)

_CACHE = {}


def build_program():
    nc = bacc.Bacc("TRN2", target_bir_lowering=False, debug=False, num_devices=8)

    qT = nc.dram_tensor("qT", [D, S], F16, kind="ExternalInput")
    kT = nc.dram_tensor("kT", [D, S], F16, kind="ExternalInput")
    vT = nc.dram_tensor("vT", [D, S], F16, kind="ExternalInput")
    maskT = nc.dram_tensor("maskT", [NVH, P, NKT, QS], I8, kind="ExternalInput")
    WqT = nc.dram_tensor("WqT", [D, DL], F16, kind="ExternalInput")
    WkT = nc.dram_tensor("WkT", [D, DL], F16, kind="ExternalInput")
    WvT = nc.dram_tensor("WvT", [D, DL], F16, kind="ExternalInput")
    WyT = nc.dram_tensor("WyT", [DL, D], F16, kind="ExternalInput")
    bq = nc.dram_tensor("bq", [P, ML], F32, kind="ExternalInput")
    bk = nc.dram_tensor("bk", [P, ML], F32, kind="ExternalInput")
    bv = nc.dram_tensor("bv", [1, DL], F32, kind="ExternalInput")
    onesd = nc.dram_tensor("onesd", [P, NKT * HL], F16, kind="ExternalInput")
    y = nc.dram_tensor("y", [S, D], F16, kind="ExternalOutput")

    qT_r = qT.rearrange("(ko p) s -> p ko s", p=P)
    kT_r = kT.rearrange("(ko p) s -> p ko s", p=P)
    vT_r = vT.rearrange("(ko p) s -> p ko s", p=P)
    WqT_r = WqT.rearrange("(ko p) m -> p ko m", p=P)
    WkT_r = WkT.rearrange("(ko p) m -> p ko m", p=P)
    WvT_r = WvT.rearrange("(ko p) m -> p ko m", p=P)
    WyT_r = WyT.rearrange("(ko p) m -> p ko m", p=P)

    def bcast_dram(ap, parts):
        return bass.AP(tensor=ap.tensor, offset=ap.offset, ap=[[0, parts]] + list(ap.ap[1:]))

    with tile.TileContext(nc) as tc:
        with (
            tc.tile_pool(name="persist", bufs=1) as persist,
            tc.tile_pool(name="w", bufs=2) as wpool,
            tc.tile_pool(name="xin", bufs=3) as xinpool,
            tc.tile_pool(name="maskp", bufs=4) as maskp,
            tc.tile_pool(name="etp", bufs=3) as etp,
            tc.tile_pool(name="outp", bufs=2) as outp,
            tc.tile_pool(name="dscr", bufs=2, space="DRAM") as dscr,
            tc.tile_pool(name="ps", bufs=2, space="PSUM") as psp,
        ):
            # ---- persistent SBUF tensors (all small now: local heads only) ----
            KT_sb = persist.tile([P, ML, S], F16)             # K^T   (8KB/part)
            QT_sb = persist.tile([P, ML, S], F16)             # Q^T   (8KB/part)
            vh = persist.tile([P, NKT, HL, DH + 1], F16)      # V     (8.1KB/part)
            attnT = persist.tile([P, ML, S], F16)             # attn^T (8KB/part)
            bq_sb = persist.tile([P, ML], F32)
            bk_sb = persist.tile([P, ML], F32)
            bv_bc = persist.tile([P, DL], F32)

            # ---- K projection: KT_sb[dm, s] = Wk_local @ k ----
            wk = wpool.tile([P, KO, DL], F16, tag="w")
            nc.sync.dma_start(out=wk[:, 0:2, :], in_=WkT_r[:, 0:2, :])
            nc.sync.dma_start(out=wk[:, 2:KO, :], in_=WkT_r[:, 2:KO, :])
            kcols = []
            for sc in range(S // QS):
                kcol = xinpool.tile([P, KO, QS], F16, tag="xcol")
                if sc == 0:
                    nc.sync.dma_start(out=kcol[:, 0:2, :], in_=kT_r[:, 0:2, 0:QS])
                    nc.sync.dma_start(out=kcol[:, 2:KO, :], in_=kT_r[:, 2:KO, 0:QS])
                else:
                    nc.sync.dma_start(out=kcol, in_=kT_r[:, :, sc * QS:(sc + 1) * QS])
                kcols.append(kcol)

            nc.scalar.dma_start(out=bq_sb, in_=bq[:])
            nc.scalar.dma_start(out=bk_sb, in_=bk[:])
            nc.scalar.dma_start(out=bv_bc, in_=bcast_dram(bv[:], P))
            # ones column of vh (softmax denominator via the AV matmul):
            # contiguous load + one strided DVE copy
            ones_row = persist.tile([P, NKT * HL], F16)
            nc.scalar.dma_start(out=ones_row, in_=onesd[:])
            nc.vector.tensor_scalar_mul(
                vh[:, :, :, DH:DH + 1].rearrange("p s h one -> p (s h one)"),
                ones_row[:], 1.0)

            for sc in range(S // QS):
                kcol = kcols[sc]
                for m in range(ML):
                    ps = psp.tile([P, QS], F32, tag="proj" if m == 0 else "att",
                                  bufs=1 if m == 0 else 3)
                    for ko in range(KO):
                        nc.tensor.matmul(
                            ps[:], wk[:, ko, m * P:(m + 1) * P], kcol[:, ko, :],
                            start=(ko == 0), stop=(ko == KO - 1))
                    nc.scalar.activation(
                        out=KT_sb[:, m, sc * QS:(sc + 1) * QS], in_=ps[:],
                        func=mybir.ActivationFunctionType.Identity,
                        bias=bk_sb[:, m:m + 1], scale=1.0)

            # ---- V projection (data-stationary): vh[s, h, dh] = v @ Wv_l.T ----
            wv = wpool.tile([P, KO, DL], F16, tag="w")
            nc.sync.dma_start(out=wv, in_=WvT_r[:])
            for sg in range(4):
                vt = xinpool.tile([P, KO, 4 * P], F16, tag="xcol")
                nc.sync.dma_start(out=vt, in_=vT_r[:, :, sg * 4 * P:(sg + 1) * 4 * P])
                for stl in range(4):
                    st = sg * 4 + stl
                    psV = psp.tile([P, QS], F32, tag="proj" if st % 2 == 0 else "att",
                                   bufs=1 if st % 2 == 0 else 3)
                    for ko in range(KO):
                        nc.tensor.matmul(psV[:, 0:DL], vt[:, ko, stl * P:(stl + 1) * P],
                                         wv[:, ko, :],
                                         start=(ko == 0), stop=(ko == KO - 1))
                    nc.vector.tensor_tensor(
                        vh[:, st, :, 0:DH],
                        psV[:, 0:DL].rearrange("p (h d) -> p h d", d=DH),
                        bv_bc.rearrange("p (h d) -> p h d", d=DH),
                        mybir.AluOpType.add)

            # ---- Q projection over all queries ----
            wq = wpool.tile([P, KO, DL], F16, tag="w")
            nc.sync.dma_start(out=wq, in_=WqT_r[:])
            qcols = []
            for sc in range(S // QS):
                qcol = xinpool.tile([P, KO, QS], F16, tag="xcol")
                nc.sync.dma_start(out=qcol, in_=qT_r[:, :, sc * QS:(sc + 1) * QS])
                qcols.append(qcol)
            for sc in range(S // QS):
                for m in range(ML):
                    ps = psp.tile([P, QS], F32, tag="proj" if m == 0 else "att",
                                  bufs=1 if m == 0 else 3)
                    for ko in range(KO):
                        nc.tensor.matmul(
                            ps[:], wq[:, ko, m * P:(m + 1) * P], qcols[sc][:, ko, :],
                            start=(ko == 0), stop=(ko == KO - 1))
                    nc.scalar.activation(
                        out=QT_sb[:, m, sc * QS:(sc + 1) * QS], in_=ps[:],
                        func=mybir.ActivationFunctionType.Identity,
                        bias=bq_sb[:, m:m + 1], scale=1.0)

            # prefetch Wy during attention
            wy = wpool.tile([P, ML, D], F16, tag="wy", bufs=1)
            nc.sync.dma_start(out=wy, in_=WyT_r[:])

            # ---- attention over 16 (local head, query block) units ----
            def emit_tail(patt, hm, hp, qc):
                # 1/denom on partition 0, then partition-broadcast to 64
                # partitions with a stride-0 SBUF->SBUF DMA (off critical path)
                dtmp = etp.tile([1, QS], F32, tag="dtmp", bufs=1)
                rtmp = etp.tile([1, QS], F32, tag="rtmp", bufs=1)
                nc.vector.tensor_scalar_mul(dtmp[:], patt[DH:DH + 1, :], 1.0)
                nc.vector.reciprocal_approx_fast(out=rtmp[:], in_=dtmp[:])
                rd = dscr.tile([1, QS], F32, tag="rd")
                nc.sync.dma_start(out=rd[:], in_=rtmp[:])
                recb = etp.tile([DH, QS], F32, tag="recb", bufs=2)
                nc.sync.dma_start(out=recb[:], in_=bcast_dram(rd[:], DH))
                nc.vector.tensor_tensor(
                    attnT[hp * DH:(hp + 1) * DH, hm, qc * QS:(qc + 1) * QS],
                    patt[0:DH, :], recb[:], mybir.AluOpType.mult)

            mbs = {}
            for v in range(2):
                mb = maskp.tile([P, NKT, QS], I8, tag="mask")
                nc.sync.dma_start(out=mb, in_=maskT[v])
                mbs[v] = mb

            pend_tail = None
            for v in range(NVH):
                h, qc = v // 4, v % 4
                hm, hp = h // 2, h % 2
                if v + 2 < NVH:
                    mb2 = maskp.tile([P, NKT, QS], I8, tag="mask")
                    nc.sync.dma_start(out=mb2, in_=maskT[v + 2])
                    mbs[v + 2] = mb2
                mb = mbs.pop(v)
                qh = QT_sb[hp * DH:(hp + 1) * DH, hm, qc * QS:(qc + 1) * QS]
                kh = KT_sb[hp * DH:(hp + 1) * DH, hm, :]
                patt = psp.tile([DH + 1, QS], F32, tag="att", bufs=3)

                # depth-2 software pipeline: AV(i) issues after scores(i+2),
                # so eT2(i) is long ready and the AV LDWEIGHTS can prefetch
                pend_av = []
                for kt2 in range(NKT // 2):
                    ka, kb = 2 * kt2, 2 * kt2 + 1
                    psS = psp.tile([P, 2, QS], F32, tag="scores")
                    nc.tensor.matmul(psS[:, 0, :], kh[:, ka * P:(ka + 1) * P], qh,
                                     start=True, stop=True)
                    nc.tensor.matmul(psS[:, 1, :], kh[:, kb * P:(kb + 1) * P], qh,
                                     start=True, stop=True, skip_group_check=True)
                    if kt2 == 1 and pend_tail is not None:
                        emit_tail(*pend_tail)
                        pend_tail = None
                    eTr = etp.tile([P, 2, QS], F16, tag="eTr", bufs=5)
                    nc.scalar.activation(out=eTr[:], in_=psS[:],
                                         func=mybir.ActivationFunctionType.Exp)
                    eT2 = etp.tile([P, 2, QS], F16, tag="eT", bufs=5)
                    # 5/3 DVE/GpSimd split: a GpSimd mult takes ~2us, more
                    # than one clamped-clock pair step, so keep them sparse
                    meng = nc.vector if kt2 not in (2, 6) else nc.gpsimd
                    meng.tensor_tensor(eT2[:], eTr[:], mb[:, ka:kb + 1, :],
                                       mybir.AluOpType.mult)
                    pend_av.append((eT2, ka, kb))
                    if len(pend_av) > 3:
                        peT, pka, pkb = pend_av.pop(0)
                        nc.tensor.matmul(patt[:], vh[:, pka, h, :], peT[:, 0, :],
                                         start=(pka == 0), stop=False)
                        nc.tensor.matmul(patt[:], vh[:, pkb, h, :], peT[:, 1, :],
                                         start=False, stop=False)
                for peT, pka, pkb in pend_av:
                    nc.tensor.matmul(patt[:], vh[:, pka, h, :], peT[:, 0, :],
                                     start=(pka == 0), stop=False)
                    nc.tensor.matmul(patt[:], vh[:, pkb, h, :], peT[:, 1, :],
                                     start=False, stop=(pkb == NKT - 1))
                pend_tail = (patt, hm, hp, qc)
            emit_tail(*pend_tail)

            # ---- Y projection: partial y (summed on host; by added there) ----
            # psum from the (now idle) scores tag so two q-chunks pipeline
            for qc16 in range(S // P):
                if qc16 % 2 == 0:
                    psY = psp.tile([P, 2, QS], F32, tag="scores")
                    psY0, psY1 = psY[:, 0, :], psY[:, 1, :]
                else:
                    psY0t = psp.tile([P, QS], F32, tag="proj", bufs=1)
                    psY1t = psp.tile([P, QS], F32, tag="att", bufs=3)
                    psY0, psY1 = psY0t[:], psY1t[:]
                for ko in range(ML):
                    lhs = attnT[:, ko, qc16 * P:(qc16 + 1) * P]
                    nc.tensor.matmul(psY0, lhs, wy[:, ko, 0:512],
                                     start=(ko == 0), stop=(ko == ML - 1))
                    nc.tensor.matmul(psY1, lhs, wy[:, ko, 512:1024],
                                     start=(ko == 0), stop=(ko == ML - 1),
                                     skip_group_check=True)
                ysb = outp.tile([P, D], F16, tag="ysb", bufs=4)
                nc.scalar.activation(out=ysb[:, 0:512], in_=psY0,
                                     func=mybir.ActivationFunctionType.Copy)
                nc.vector.tensor_scalar_mul(ysb[:, 512:1024], psY1, 1.0)
                nc.sync.dma_start(out=y[qc16 * P:(qc16 + 1) * P, :], in_=ysb[:])

    nc.compile()
    return nc


def prep_inputs(queries, keys, values, mask, Wq, bq, Wk, bk, Wv, bv, Wy, by,
                bq2, bk2, bv2, by2):
    f = np.float32
    WqT_f = (Wq.astype(f) / 8.0).T
    WkT_f = Wk.astype(f).T
    WvT_f = Wv.astype(f).T
    WyT_f = Wy.astype(f).T
    bq_f = (bq + bq2).astype(f) / 8.0
    bk_f = (bk + bk2).astype(f)
    bv_f = (bv + bv2).astype(f)

    onesd = np.ones((P, NKT * HL), dtype=NP16)

    qT = [np.ascontiguousarray(queries[b].astype(f).T.astype(NP16)) for b in range(B)]
    kT = [np.ascontiguousarray(keys[b].astype(f).T.astype(NP16)) for b in range(B)]
    vT = [np.ascontiguousarray(values[b].astype(f).T.astype(NP16)) for b in range(B)]

    in_maps = []
    for c in range(8):
        b, g = c // 4, c % 4
        lo, hi = g * DL, (g + 1) * DL
        # maskT[v= h*4+qc, p, kt, ql] = mask[b, 4g+h, qc*512+ql, kt*128+p]
        mT = mask[b, 4 * g:4 * g + HL].astype(np.int8)          # [h, q, s]
        mT = mT.reshape(HL, 4, QS, NKT, P)                      # [h, qc, ql, kt, p]
        mT = np.ascontiguousarray(mT.transpose(0, 1, 4, 3, 2))  # [h, qc, p, kt, ql]
        mT = mT.reshape(NVH, P, NKT, QS)
        in_maps.append({
            "qT": qT[b], "kT": kT[b], "vT": vT[b],
            "maskT": mT,
            "WqT": np.ascontiguousarray(WqT_f[:, lo:hi].astype(NP16)),
            "WkT": np.ascontiguousarray(WkT_f[:, lo:hi].astype(NP16)),
            "WvT": np.ascontiguousarray(WvT_f[:, lo:hi].astype(NP16)),
            "WyT": np.ascontiguousarray(WyT_f[lo:hi, :].astype(NP16)),
            "bq": np.ascontiguousarray(bq_f[lo:hi].reshape(ML, P).T),
            "bk": np.ascontiguousarray(bk_f[lo:hi].reshape(ML, P).T),
            "bv": np.ascontiguousarray(bv_f[lo:hi][None, :]),
            "onesd": onesd,
        })
    return in_maps


def kernel(**inputs):
    if "nc" not in _CACHE:
        _CACHE["nc"] = build_program()
    nc = _CACHE["nc"]
    in_maps = prep_inputs(**inputs)
    res = run_bass_kernel_spmd(nc, in_maps, core_ids=list(range(8)))
    out = np.zeros((B, S, D), dtype=np.float32)
    for c in range(8):
        b = c // 4
        out[b] += res.results[c]["y"].astype(np.float32)
    out += (inputs["by"] + inputs["by2"]).astype(np.float32)
    return out


# revision 70
# speedup vs baseline: 1.0048x; 1.0048x over previous
"""MultiHeadAttention Trainium2 Bass kernel (v3, fp16, head-sharded).

Model: B=2, S=2048, D_MODEL=1024, H=16 heads, Dh=64.
  q/k/v = x @ W.T + b ; scores = (q k^T)/8 masked-softmax ; out = w @ v ; y = out @ Wy.T + by

Sharding (per the tensor-parallel head sharding): core c handles batch
b = c // 4 and head group g = c % 4 (4 heads), over ALL 2048 queries.
Wq/Wk/Wv are column-sharded, Wy row-sharded: each core emits a PARTIAL
y (fp16); kernel() sums the 4 partials per batch on the host and adds by.
This avoids the 4x duplicated K/V projection of query-sharding and cuts
per-core matmul cycles from ~591k to ~394k — which matters doubly here
because the board clamps the PE clock to 1.2GHz after ~150us of
sustained matmul activity.

Other design points:
  - all matmul operands fp16 (fp32 PSUM accum); V stays in SBUF
  - mask applied multiplicatively AFTER exp (exp reads PSUM on Act,
    fp16 mask-multiply split across DVE and GpSimd)
  - per-(head, query-block) softmax normalization: denominator row moved
    to partition 0, reciprocal_approx_fast, broadcast to 64 partitions
    via a ones-vector matmul, multiply fused with the PSUM eviction;
    the whole tail is deferred behind the next block's first scores
    matmuls so it never stalls the PE queue
"""

import numpy as np

import concourse.bass as bass
import concourse.mybir as mybir
import concourse.tile as tile
from concourse import bacc
from concourse.bass_utils import run_bass_kernel_spmd

F32 = mybir.dt.float32
F16 = mybir.dt.float16
I8 = mybir.dt.int8
NP16 = np.float16

B, S, D, H, DH = 2, 2048, 1024, 16, 64
QS = 512
P = 128
KO = D // P    # 8 contraction chunks over D
NKT = S // P   # 16 key chunks
HL = 4         # local heads per core
DL = HL * DH   # 256 local head dims
ML = DL // P   # 2 local m-blocks
NVH = HL * (S // QS)  # 16 virtual heads = (local head, query block)

_CACHE = {}


def build_program():
    nc = bacc.Bacc("TRN2", target_bir_lowering=False, debug=False, num_devices=8)

    qT = nc.dram_tensor("qT", [D, S], F16, kind="ExternalInput")
    kT = nc.dram_tensor("kT", [D, S], F16, kind="ExternalInput")
    vT = nc.dram_tensor("vT", [D, S], F16, kind="ExternalInput")
    maskT = nc.dram_tensor("maskT", [NVH, P, NKT, QS], I8, kind="ExternalInput")
    WqT = nc.dram_tensor("WqT", [D, DL], F16, kind="ExternalInput")
    WkT = nc.dram_tensor("WkT", [D, DL], F16, kind="ExternalInput")
    WvT = nc.dram_tensor("WvT", [D, DL], F16, kind="ExternalInput")
    WyT = nc.dram_tensor("WyT", [DL, D], F16, kind="ExternalInput")
    bq = nc.dram_tensor("bq", [P, ML], F32, kind="ExternalInput")
    bk = nc.dram_tensor("bk", [P, ML], F32, kind="ExternalInput")
    bv = nc.dram_tensor("bv", [1, DL], F32, kind="ExternalInput")
    onesd = nc.dram_tensor("onesd", [P, NKT * HL], F16, kind="ExternalInput")
    y = nc.dram_tensor("y", [S, D], F16, kind="ExternalOutput")

    qT_r = qT.rearrange("(ko p) s -> p ko s", p=P)
    kT_r = kT.rearrange("(ko p) s -> p ko s", p=P)
    vT_r = vT.rearrange("(ko p) s -> p ko s", p=P)
    WqT_r = WqT.rearrange("(ko p) m -> p ko m", p=P)
    WkT_r = WkT.rearrange("(ko p) m -> p ko m", p=P)
    WvT_r = WvT.rearrange("(ko p) m -> p ko m", p=P)
    WyT_r = WyT.rearrange("(ko p) m -> p ko m", p=P)

    def bcast_dram(ap, parts):
        return bass.AP(tensor=ap.tensor, offset=ap.offset, ap=[[0, parts]] + list(ap.ap[1:]))

    with tile.TileContext(nc) as tc:
        with (
            tc.tile_pool(name="persist", bufs=1) as persist,
            tc.tile_pool(name="w", bufs=2) as wpool,
            tc.tile_pool(name="xin", bufs=3) as xinpool,
            tc.tile_pool(name="maskp", bufs=4) as maskp,
            tc.tile_pool(name="etp", bufs=3) as etp,
            tc.tile_pool(name="outp", bufs=2) as outp,
            tc.tile_pool(name="dscr", bufs=2, space="DRAM") as dscr,
            tc.tile_pool(name="ps", bufs=2, space="PSUM") as psp,
        ):
            # ---- persistent SBUF tensors (all small now: local heads only) ----
            KT_sb = persist.tile([P, ML, S], F16)             # K^T   (8KB/part)
            QT_sb = persist.tile([P, ML, S], F16)             # Q^T   (8KB/part)
            vh = persist.tile([P, NKT, HL, DH + 1], F16)      # V     (8.1KB/part)
            attnT = persist.tile([P, ML, S], F16)             # attn^T (8KB/part)
            bq_sb = persist.tile([P, ML], F32)
            bk_sb = persist.tile([P, ML], F32)
            bv_bc = persist.tile([P, DL], F32)

            # ---- K projection: KT_sb[dm, s] = Wk_local @ k ----
            wk = wpool.tile([P, KO, DL], F16, tag="w")
            nc.sync.dma_start(out=wk[:, 0:2, :], in_=WkT_r[:, 0:2, :])
            nc.sync.dma_start(out=wk[:, 2:KO, :], in_=WkT_r[:, 2:KO, :])
            kcols = []
            for sc in range(S // QS):
                kcol = xinpool.tile([P, KO, QS], F16, tag="xcol")
                if sc == 0:
                    nc.sync.dma_start(out=kcol[:, 0:2, :], in_=kT_r[:, 0:2, 0:QS])
                    nc.sync.dma_start(out=kcol[:, 2:KO, :], in_=kT_r[:, 2:KO, 0:QS])
                else:
                    nc.sync.dma_start(out=kcol, in_=kT_r[:, :, sc * QS:(sc + 1) * QS])
                kcols.append(kcol)

            nc.scalar.dma_start(out=bq_sb, in_=bq[:])
            nc.scalar.dma_start(out=bk_sb, in_=bk[:])
            nc.scalar.dma_start(out=bv_bc, in_=bcast_dram(bv[:], P))
            # ones column of vh (softmax denominator via the AV matmul):
            # contiguous load + one strided DVE copy
            ones_row = persist.tile([P, NKT * HL], F16)
            nc.scalar.dma_start(out=ones_row, in_=onesd[:])
            nc.vector.tensor_scalar_mul(
                vh[:, :, :, DH:DH + 1].rearrange("p s h one -> p (s h one)"),
                ones_row[:], 1.0)

            for sc in range(S // QS):
                kcol = kcols[sc]
                for m in range(ML):
                    ps = psp.tile([P, QS], F32, tag="proj" if m == 0 else "att",
                                  bufs=1 if m == 0 else 3)
                    for ko in range(KO):
                        nc.tensor.matmul(
                            ps[:], wk[:, ko, m * P:(m + 1) * P], kcol[:, ko, :],
                            start=(ko == 0), stop=(ko == KO - 1))
                    nc.scalar.activation(
                        out=KT_sb[:, m, sc * QS:(sc + 1) * QS], in_=ps[:],
                        func=mybir.ActivationFunctionType.Identity,
                        bias=bk_sb[:, m:m + 1], scale=1.0)

            # ---- V projection (data-stationary): vh[s, h, dh] = v @ Wv_l.T ----
            wv = wpool.tile([P, KO, DL], F16, tag="w")
            nc.sync.dma_start(out=wv, in_=WvT_r[:])
            for sg in range(4):
                vt = xinpool.tile([P, KO, 4 * P], F16, tag="xcol")
                nc.sync.dma_start(out=vt, in_=vT_r[:, :, sg * 4 * P:(sg + 1) * 4 * P])
                for stl in range(4):
                    st = sg * 4 + stl
                    psV = psp.tile([P, QS], F32, tag="proj" if st % 2 == 0 else "att",
                                   bufs=1 if st % 2 == 0 else 3)
                    for ko in range(KO):
                        nc.tensor.matmul(psV[:, 0:DL], vt[:, ko, stl * P:(stl + 1) * P],
                                         wv[:, ko, :],
                                         start=(ko == 0), stop=(ko == KO - 1))
                    nc.vector.tensor_tensor(
                        vh[:, st, :, 0:DH],
                        psV[:, 0:DL].rearrange("p (h d) -> p h d", d=DH),
                        bv_bc.rearrange("p (h d) -> p h d", d=DH),
                        mybir.AluOpType.add)

            # ---- Q projection over all queries ----
            wq = wpool.tile([P, KO, DL], F16, tag="w")
            nc.sync.dma_start(out=wq, in_=WqT_r[:])
            qcols = []
            for sc in range(S // QS):
                qcol = xinpool.tile([P, KO, QS], F16, tag="xcol")
                nc.sync.dma_start(out=qcol, in_=qT_r[:, :, sc * QS:(sc + 1) * QS])
                qcols.append(qcol)
            for sc in range(S // QS):
                for m in range(ML):
                    ps = psp.tile([P, QS], F32, tag="proj" if m == 0 else "att",
                                  bufs=1 if m == 0 else 3)
                    for ko in range(KO):
                        nc.tensor.matmul(
                            ps[:], wq[:, ko, m * P:(m + 1) * P], qcols[sc][:, ko, :],
                            start=(ko == 0), stop=(ko == KO - 1))
                    nc.scalar.activation(
                        out=QT_sb[:, m, sc * QS:(sc + 1) * QS], in_=ps[:],
                        func=mybir.ActivationFunctionType.Identity,
                        bias=bq_sb[:, m:m + 1], scale=1.0)

            # prefetch Wy during attention
            wy = wpool.tile([P, ML, D], F16, tag="wy", bufs=1)
            nc.sync.dma_start(out=wy, in_=WyT_r[:])

            # ---- attention over 16 (local head, query block) units ----
            def emit_tail(patt, hm, hp, qc):
                # 1/denom on partition 0, then partition-broadcast to 64
                # partitions with a stride-0 SBUF->SBUF DMA (off critical path)
                dtmp = etp.tile([1, QS], F32, tag="dtmp", bufs=1)
                rtmp = etp.tile([1, QS], F32, tag="rtmp", bufs=1)
                nc.vector.tensor_scalar_mul(dtmp[:], patt[DH:DH + 1, :], 1.0)
                nc.vector.reciprocal_approx_fast(out=rtmp[:], in_=dtmp[:])
                rd = dscr.tile([1, QS], F32, tag="rd")
                nc.sync.dma_start(out=rd[:], in_=rtmp[:])
                recb = etp.tile([DH, QS], F32, tag="recb", bufs=2)
                nc.sync.dma_start(out=recb[:], in_=bcast_dram(rd[:], DH))
                nc.vector.tensor_tensor(
                    attnT[hp * DH:(hp + 1) * DH, hm, qc * QS:(qc + 1) * QS],
                    patt[0:DH, :], recb[:], mybir.AluOpType.mult)

            mbs = {}
            for v in range(2):
                mb = maskp.tile([P, NKT, QS], I8, tag="mask")
                nc.sync.dma_start(out=mb, in_=maskT[v])
                mbs[v] = mb

            pend_tail = None
            for v in range(NVH):
                h, qc = v // 4, v % 4
                hm, hp = h // 2, h % 2
                if v + 2 < NVH:
                    mb2 = maskp.tile([P, NKT, QS], I8, tag="mask")
                    nc.sync.dma_start(out=mb2, in_=maskT[v + 2])
                    mbs[v + 2] = mb2
                mb = mbs.pop(v)
                qh = QT_sb[hp * DH:(hp + 1) * DH, hm, qc * QS:(qc + 1) * QS]
                kh = KT_sb[hp * DH:(hp + 1) * DH, hm, :]
                patt = psp.tile([DH + 1, QS], F32, tag="att", bufs=3)

                # depth-2 software pipeline: AV(i) issues after scores(i+2),
                # so eT2(i) is long ready and the AV LDWEIGHTS can prefetch
                pend_av = []
                for kt2 in range(NKT // 2):
                    ka, kb = 2 * kt2, 2 * kt2 + 1
                    psS = psp.tile([P, 2, QS], F32, tag="scores")
                    nc.tensor.matmul(psS[:, 0, :], kh[:, ka * P:(ka + 1) * P], qh,
                                     start=True, stop=True)
                    nc.tensor.matmul(psS[:, 1, :], kh[:, kb * P:(kb + 1) * P], qh,
                                     start=True, stop=True, skip_group_check=True)
                    if kt2 == 1 and pend_tail is not None:
                        emit_tail(*pend_tail)
                        pend_tail = None
                    eTr = etp.tile([P, 2, QS], F16, tag="eTr", bufs=5)
                    nc.scalar.activation(out=eTr[:], in_=psS[:],
                                         func=mybir.ActivationFunctionType.Exp)
                    eT2 = etp.tile([P, 2, QS], F16, tag="eT", bufs=5)
                    # 5/3 DVE/GpSimd split: a GpSimd mult takes ~2us, more
                    # than one clamped-clock pair step, so keep them sparse
                    meng = nc.vector if kt2 not in (2, 6) else nc.gpsimd
                    meng.tensor_tensor(eT2[:], eTr[:], mb[:, ka:kb + 1, :],
                                       mybir.AluOpType.mult)
                    pend_av.append((eT2, ka, kb))
                    if len(pend_av) > 3:
                        peT, pka, pkb = pend_av.pop(0)
                        nc.tensor.matmul(patt[:], vh[:, pka, h, :], peT[:, 0, :],
                                         start=(pka == 0), stop=False)
                        nc.tensor.matmul(patt[:], vh[:, pkb, h, :], peT[:, 1, :],
                                         start=False, stop=False)
                for peT, pka, pkb in pend_av:
                    nc.tensor.matmul(patt[:], vh[:, pka, h, :], peT[:, 0, :],
                                     start=(pka == 0), stop=False)
                    nc.tensor.matmul(patt[:], vh[:, pkb, h, :], peT[:, 1, :],
                                     start=False, stop=(pkb == NKT - 1))
                pend_tail = (patt, hm, hp, qc)
            emit_tail(*pend_tail)

            # ---- Y projection: partial y (summed on host; by added there) ----
            # psum from the (now idle) scores tag so two q-chunks pipeline
            for qc16 in range(S // P):
                if qc16 % 2 == 0:
                    psY = psp.tile([P, 2, QS], F32, tag="scores")
                    psY0, psY1 = psY[:, 0, :], psY[:, 1, :]
                else:
                    psY0t = psp.tile([P, QS], F32, tag="proj", bufs=1)
                    psY1t = psp.tile([P, QS], F32, tag="att", bufs=3)
                    psY0, psY1 = psY0t[:], psY1t[:]
                for ko in range(ML):
                    lhs = attnT[:, ko, qc16 * P:(qc16 + 1) * P]
                    nc.tensor.matmul(psY0, lhs, wy[:, ko, 0:512],
                                     start=(ko == 0), stop=(ko == ML - 1))
                    nc.tensor.matmul(psY1, lhs, wy[:, ko, 512:1024],
                                     start=(ko == 0), stop=(ko == ML - 1),
                                     skip_group_check=True)
                ysb = outp.tile([P, D], F16, tag="ysb", bufs=4)
                nc.scalar.activation(out=ysb[:, 0:512], in_=psY0,
                                     func=mybir.ActivationFunctionType.Copy)
                nc.vector.tensor_scalar_mul(ysb[:, 512:1024], psY1, 1.0)
                nc.sync.dma_start(out=y[qc16 * P:(qc16 + 1) * P, :], in_=ysb[:])

    nc.compile()
    return nc


def prep_inputs(queries, keys, values, mask, Wq, bq, Wk, bk, Wv, bv, Wy, by,
                bq2, bk2, bv2, by2):
    f = np.float32
    WqT_f = (Wq.astype(f) / 8.0).T
    WkT_f = Wk.astype(f).T
    WvT_f = Wv.astype(f).T
    WyT_f = Wy.astype(f).T
    bq_f = (bq + bq2).astype(f) / 8.0
    bk_f = (bk + bk2).astype(f)
    bv_f = (bv + bv2).astype(f)

    onesd = np.ones((P, NKT * HL), dtype=NP16)

    qT = [np.ascontiguousarray(queries[b].astype(f).T.astype(NP16)) for b in range(B)]
    kT = [np.ascontiguousarray(keys[b].astype(f).T.astype(NP16)) for b in range(B)]
    vT = [np.ascontiguousarray(values[b].astype(f).T.astype(NP16)) for b in range(B)]

    in_maps = []
    for c in range(8):
        b, g = c // 4, c % 4
        lo, hi = g * DL, (g + 1) * DL
        # maskT[v= h*4+qc, p, kt, ql] = mask[b, 4g+h, qc*512+ql, kt*128+p]
        mT = mask[b, 4 * g:4 * g + HL].astype(np.int8)          # [h, q, s]
        mT = mT.reshape(HL, 4, QS, NKT, P)                      # [h, qc, ql, kt, p]
        mT = np.ascontiguousarray(mT.transpose(0, 1, 4, 3, 2))  # [h, qc, p, kt, ql]
        mT = mT.reshape(NVH, P, NKT, QS)
        in_maps.append({
            "qT": qT[b], "kT": kT[b], "vT": vT[b],
            "maskT": mT,
            "WqT": np.ascontiguousarray(WqT_f[:, lo:hi].astype(NP16)),
            "WkT": np.ascontiguousarray(WkT_f[:, lo:hi].astype(NP16)),
            "WvT": np.ascontiguousarray(WvT_f[:, lo:hi].astype(NP16)),
            "WyT": np.ascontiguousarray(WyT_f[lo:hi, :].astype(NP16)),
            "bq": np.ascontiguousarray(bq_f[lo:hi].reshape(ML, P).T),
            "bk": np.ascontiguousarray(bk_f[lo:hi].reshape(ML, P).T),
            "bv": np.ascontiguousarray(bv_f[lo:hi][None, :]),
            "onesd": onesd,
        })
    return in_maps


def kernel(**inputs):
    if "nc" not in _CACHE:
        _CACHE["nc"] = build_program()
    nc = _CACHE["nc"]
    in_maps = prep_inputs(**inputs)
    res = run_bass_kernel_spmd(nc, in_maps, core_ids=list(range(8)))
    out = np.zeros((B, S, D), dtype=np.float32)
    for c in range(8):
        b = c // 4
        out[b] += res.results[c]["y"].astype(np.float32)
    out += (inputs["by"] + inputs["by2"]).astype(np.float32)
    return out
